# revision 1
# baseline (speedup 1.0000x reference)
"""Multi-head causal self-attention (B=4, S=2048, E=1024, H=16, D=64) on 8 TRN2 cores.

Sharding: data-parallel over batch (4 batches x 2 cores each); within a batch
pair, queries are split causally-balanced (zigzag q-blocks) so both cores do
equal attention work with zero cross-core communication.  Each core computes
K/V for the full sequence of its batch (all heads), Q for its own 1024 rows,
causal attention, and the output projection for its own rows.  The host only
shards inputs and scatters the disjoint output rows back.

Compute dtype: float16 matmul operands, fp32 PSUM accumulation, exp on ACT in
fp32 with per-(span,block) scale/bias vectors implementing block-level causal
kills; triangular diagonal masks are additive f32 tiles from the host.
"""

import os
import numpy as np

import concourse.mybir as mybir
import concourse.tile as tile
import concourse.bacc as bacc
from concourse.bass_utils import run_bass_kernel_spmd

B, S, E, H, D = 4, 2048, 1024, 16, 64
KB = S // 128            # 16 kv blocks of 128
NSPAN = 4                # q spans of 256 per core
F32 = mybir.dt.float32
F16 = mybir.dt.float16

# q-block (128-row) assignment per half, grouped into 4 spans of 2 blocks.
OWN_BLOCKS = {
    0: [[0, 1], [6, 7], [8, 9], [14, 15]],
    1: [[2, 3], [4, 5], [10, 11], [12, 13]],
}
BOUNDS = [4, 8, 12, 16]          # uniform kv-block bound per span slot
MASK_OFFS = [0, 128, 384, 512]   # packed col offsets of the 4 masked blocks
MASK_W = [128, 256, 128, 256]    # widths (m%2+1)*128


def own_rows(half):
    return np.concatenate(
        [np.arange(b * 128, b * 128 + 128) for sp in OWN_BLOCKS[half] for b in sp]
    )


def _build_nc():
    nc = bacc.Bacc("TRN2", target_bir_lowering=False, debug=False, num_devices=8)
    xt = nc.dram_tensor("xt", [E, S], F32, kind="ExternalInput")
    xqt = nc.dram_tensor("xqt", [E, S // 2], F32, kind="ExternalInput")
    wqt = nc.dram_tensor("wqt", [E, E], F32, kind="ExternalInput")
    wkt = nc.dram_tensor("wkt", [E, E], F32, kind="ExternalInput")
    wvt = nc.dram_tensor("wvt", [E, E], F32, kind="ExternalInput")
    wp = nc.dram_tensor("wp", [E, E], F32, kind="ExternalInput")
    bpv = nc.dram_tensor("bpv", [1, E], F32, kind="ExternalInput")
    masks = nc.dram_tensor("masks", [128, NSPAN, 768], F32, kind="ExternalInput")
    scv_d = nc.dram_tensor("scv", [128, NSPAN, KB], F32, kind="ExternalInput")
    biv_d = nc.dram_tensor("biv", [128, NSPAN, KB], F32, kind="ExternalInput")
    out = nc.dram_tensor("out", [S // 2, E], F32, kind="ExternalOutput")

    xt_r = xt.rearrange("(c p) s -> p c s", p=128)
    xqt_r = xqt.rearrange("(c p) s -> p c s", p=128)
    wqt_r = wqt.rearrange("(c p) n -> p c n", p=128)
    wkt_r = wkt.rearrange("(c p) n -> p c n", p=128)
    wvt_r = wvt.rearrange("(c p) n -> p c n", p=128)
    wp_r = wp.rearrange("(c p) n -> p c n", p=128)

    with tile.TileContext(nc) as tc:
        with tc.tile_pool(name="persist", bufs=1) as pers, \
             tc.tile_pool(name="ph1", bufs=2) as ph1, \
             tc.tile_pool(name="ph2", bufs=3) as ph2, \
             tc.tile_pool(name="ph2c", bufs=4) as ph2c, \
             tc.tile_pool(name="ph3", bufs=2) as ph3, \
             tc.tile_pool(name="psA", bufs=2, space="PSUM") as psA, \
             tc.tile_pool(name="psS", bufs=3, space="PSUM") as psS, \
             tc.tile_pool(name="psC", bufs=1, space="PSUM") as psC, \
             tc.tile_pool(name="psO", bufs=1, space="PSUM") as psO, \
             tc.tile_pool(name="dram", bufs=4, space="DRAM") as dram:

            # persistent split tiles: chunk granularity so attention/proj can
            # start as soon as their inputs exist.
            KT = [[pers.tile([128, 512], F16, tag=f"kt{i}_{c}", name=f"kt{i}_{c}")
                   for c in range(4)] for i in range(8)]
            QT = [[pers.tile([128, 512], F16, tag=f"qt{i}_{c}", name=f"qt{i}_{c}")
                   for c in range(2)] for i in range(8)]
            VA = [pers.tile([128, 4, H, 65], F16, tag=f"va{c}", name=f"va{c}")
                  for c in range(4)]
            CN = [[pers.tile([128, 256], F16, tag=f"cn{i}_{c}", name=f"cn{i}_{c}")
                   for c in range(NSPAN)] for i in range(8)]

            for c in range(4):
                nc.gpsimd.memset(VA[c][:, :, :, 64:65], 1.0)

            mk = pers.tile([128, NSPAN, 768], F32)
            nc.sync.dma_start(mk[:], masks[:])
            scv = pers.tile([128, NSPAN, KB], F32)
            nc.sync.dma_start(scv[:], scv_d[:])
            biv = pers.tile([128, NSPAN, KB], F32)
            nc.sync.dma_start(biv[:], biv_d[:])
            wpt = pers.tile([128, 8, E], F16)
            nc.gpsimd.dma_start(wpt[:], wp_r[:])
            bpb = pers.tile([128, E], F32)
            nc.sync.dma_start(bpb[:], bpv[0:1, :].to_broadcast((128, E)))

            def emit_qt(qh):
                xqs = ph1.tile([128, 8, 512], F16, tag="xts")
                nc.gpsimd.dma_start(xqs[:], xqt_r[:, :, qh * 512:(qh + 1) * 512])
                for hp in range(8):
                    wq_t = ph1.tile([128, 8, 128], F16, tag="wk")
                    nc.gpsimd.dma_start(wq_t[:], wqt_r[:, :, hp * 128:(hp + 1) * 128])
                    qps = psA.tile([128, 512], F32, tag="pps")
                    for ec in range(8):
                        nc.tensor.matmul(qps[:], wq_t[:, ec, :], xqs[:, ec, :],
                                         start=(ec == 0), stop=(ec == 7))
                    nc.vector.tensor_copy(QT[hp][qh][:], qps[:])

            def emit_kv(sp4):
                xts = ph1.tile([128, 8, 512], F16, tag="xts")
                nc.gpsimd.dma_start(xts[:], xt_r[:, :, sp4 * 512:(sp4 + 1) * 512])
                for hp in range(8):
                    wk_t = ph1.tile([128, 8, 128], F16, tag="wk")
                    nc.gpsimd.dma_start(wk_t[:], wkt_r[:, :, hp * 128:(hp + 1) * 128])
                    kps = psA.tile([128, 512], F32, tag="pps")
                    for ec in range(8):
                        nc.tensor.matmul(kps[:], wk_t[:, ec, :], xts[:, ec, :],
                                         start=(ec == 0), stop=(ec == 7))
                    nc.vector.tensor_copy(KT[hp][sp4][:], kps[:])
                for hh in range(2):
                    wv_t = ph1.tile([128, 8, 512], F16, tag="wv")
                    nc.gpsimd.dma_start(wv_t[:], wvt_r[:, :, hh * 512:(hh + 1) * 512])
                    for j in range(4):
                        vps = psA.tile([128, 512], F32, tag="pps")
                        for ec in range(8):
                            nc.tensor.matmul(vps[:], xts[:, ec, j * 128:(j + 1) * 128],
                                             wv_t[:, ec, :],
                                             start=(ec == 0), stop=(ec == 7))
                        for hl in range(8):
                            h = hh * 8 + hl
                            nc.vector.tensor_copy(VA[sp4][:, j, h, 0:64],
                                                  vps[:, hl * 64:hl * 64 + 64])

            def emit_span(s):
                Bs = BOUNDS[s]
                q0 = s * 256
                qh, qo = s // 2, (s % 2) * 256
                for gg in range(8):
                    # group of 2 same-parity heads: j = gg // 2, p = gg % 2
                    # heads (4j+p, 4j+2+p) -> KT/QT pair rows p*64..p*64+64
                    j, p = gg // 2, gg % 2
                    hA, hB = 4 * j + p, 4 * j + 2 + p
                    r0 = p * 64
                    tp = (64, 0) if p else None
                    cpsA = psC.tile([65, 256], F32, tag="cpsA")
                    cpsB = psC.tile([65, 256], F32, tag="cpsB")
                    for kb in range(Bs):
                        sps = psS.tile([128, 2, 256], F32, tag="sps")
                        c4, k0 = kb // 4, (kb % 4) * 128
                        nc.tensor.matmul(sps[:, 0, :],
                                         KT[2 * j][c4][r0:r0 + 64, k0:k0 + 128],
                                         QT[2 * j][qh][r0:r0 + 64, qo:qo + 256],
                                         start=True, stop=True, tile_position=tp)
                        nc.tensor.matmul(sps[:, 1, :],
                                         KT[2 * j + 1][c4][r0:r0 + 64, k0:k0 + 128],
                                         QT[2 * j + 1][qh][r0:r0 + 64, qo:qo + 256],
                                         start=True, stop=True, tile_position=tp)
                        m = kb - (Bs - 4)
                        if m >= 0:
                            w = MASK_W[m]
                            off = MASK_OFFS[m]
                            mkap = mk[:, s, off:off + w] \
                                .rearrange("p (o w) -> p o w", o=1) \
                                .to_broadcast((128, 2, w))
                            nc.vector.tensor_add(sps[:, 0:2, 0:w], sps[:, 0:2, 0:w], mkap)
                        pt = ph2.tile([128, 2, 256], F16, tag="pt")
                        nc.scalar.activation(pt[:], sps[:], mybir.ActivationFunctionType.Exp,
                                             scale=scv[:, s, kb:kb + 1],
                                             bias=biv[:, s, kb:kb + 1])
                        st, en = (kb == 0), (kb == Bs - 1)
                        nc.tensor.matmul(cpsA[:], VA[c4][:, kb % 4, hA, :], pt[:, 0, :],
                                         start=st, stop=en)
                        nc.tensor.matmul(cpsB[:], VA[c4][:, kb % 4, hB, :], pt[:, 1, :],
                                         start=st, stop=en)
                    # stage out of PSUM quickly, then normalize from SBUF
                    ctxu = ph2c.tile([65, 2, 256], F32, tag="ctxu")
                    nc.vector.tensor_copy(ctxu[:, 0, :], cpsA[:])
                    nc.vector.tensor_copy(ctxu[:, 1, :], cpsB[:])
                    rs = ph2c.tile([1, 2, 256], F32, tag="rs")
                    nc.vector.tensor_copy(rs[0:1, 0, :], ctxu[64:65, 0, :])
                    nc.vector.tensor_copy(rs[0:1, 1, :], ctxu[64:65, 1, :])
                    rr = ph2c.tile([1, 2, 256], F32, tag="rr")
                    nc.vector.reciprocal_approx_fast(rr[:], rs[:])
                    rd = dram.tile([1, 2, 256], F32, tag="rd")
                    nc.sync.dma_start(rd[:], rr[:])
                    for i, h in ((0, hA), (1, hB)):
                        bct = ph2c.tile([64, 256], F32, tag="bct")
                        nc.sync.dma_start(bct[:], rd[0:1, i, :].to_broadcast((64, 256)))
                        hp2, rr0 = h // 2, (h % 2) * 64
                        nc.vector.tensor_mul(CN[hp2][s][rr0:rr0 + 64, :],
                                             ctxu[0:64, i, :], bct[:])

            def emit_proj(s):
                for qb in (2 * s, 2 * s + 1):
                    for eo in range(2):
                        ops = psO.tile([128, 512], F32, tag="ops")
                        for c in range(8):
                            nc.tensor.matmul(ops[:],
                                             CN[c][s][:, (qb % 2) * 128:(qb % 2) * 128 + 128],
                                             wpt[:, c, eo * 512:(eo + 1) * 512],
                                             start=(c == 0), stop=(c == 7))
                        ot = ph3.tile([128, 512], F32, tag="ot")
                        nc.vector.tensor_add(ot[:], ops[:], bpb[:, eo * 512:(eo + 1) * 512])
                        nc.sync.dma_start(out[qb * 128:(qb + 1) * 128,
                                              eo * 512:(eo + 1) * 512], ot[:])

            emit_qt(0)
            emit_kv(0)
            emit_span(0)
            emit_proj(0)
            emit_kv(1)
            emit_span(1)
            emit_proj(1)
            emit_qt(1)
            emit_kv(2)
            emit_span(2)
            emit_proj(2)
            emit_kv(3)
            emit_span(3)
            emit_proj(3)
    nc.compile()
    return nc


_NC_CACHE = None


def _host_side_tables(half):
    """Triangular masks, scale and bias vectors for one core half."""
    mask = np.zeros((128, NSPAN, 768), np.float32)
    scv = np.zeros((128, NSPAN, KB), np.float32)
    biv = np.zeros((128, NSPAN, KB), np.float32)
    for s in range(NSPAN):
        Bs = BOUNDS[s]
        gmax = OWN_BLOCKS[half][s][1]
        for kb in range(Bs):
            if kb > gmax:
                scv[:, s, kb] = 0.0
                biv[:, s, kb] = -30.0
            else:
                scv[:, s, kb] = 1.0 / np.sqrt(D)
                biv[:, s, kb] = 0.0
        for m in range(4):
            kb = Bs - 4 + m
            w = MASK_W[m]
            off = MASK_OFFS[m]
            nqb = m % 2 + 1   # q-blocks covered by this mask
            for jj in range(nqb):
                g = OWN_BLOCKS[half][s][jj]
                kpos = kb * 128 + np.arange(128)[:, None]
                qpos = g * 128 + np.arange(128)[None, :]
                mask[:, s, off + jj * 128: off + (jj + 1) * 128] = \
                    np.where(qpos >= kpos, 0.0, -240.0)
    return mask, scv, biv


def kernel(x, Wq, Wk, Wv, Wp, bp):
    global _NC_CACHE
    x = np.asarray(x, np.float32)
    Wq = np.asarray(Wq, np.float32)
    Wk = np.asarray(Wk, np.float32)
    Wv = np.asarray(Wv, np.float32)
    Wp = np.asarray(Wp, np.float32)
    bp = np.asarray(bp, np.float32)

    if _NC_CACHE is None:
        _NC_CACHE = _build_nc()
    nc = _NC_CACHE

    wqt = np.ascontiguousarray(Wq.transpose(1, 0, 2).reshape(E, E))
    wkt = np.ascontiguousarray(Wk.transpose(1, 0, 2).reshape(E, E))
    wvt = np.ascontiguousarray(Wv.transpose(1, 0, 2).reshape(E, E))
    wp_c = np.ascontiguousarray(Wp)
    bpv = bp.reshape(1, E)
    tables = {h: _host_side_tables(h) for h in (0, 1)}
    rows = {h: own_rows(h) for h in (0, 1)}

    in_maps = []
    for c in range(8):
        b, h = c // 2, c % 2
        xb = x[b]
        mask, scv, biv = tables[h]
        in_maps.append({
            "xt": np.ascontiguousarray(xb.T),
            "xqt": np.ascontiguousarray(xb[rows[h]].T),
            "wqt": wqt, "wkt": wkt, "wvt": wvt, "wp": wp_c, "bpv": bpv,
            "masks": mask, "scv": scv, "biv": biv,
        })

    trace = bool(os.environ.get("BASS_ATTN_TRACE"))
    res = run_bass_kernel_spmd(nc, in_maps, core_ids=list(range(8)), trace=trace)
    if trace and res.exec_time_ns is not None:
        print(f"HW exec time: {res.exec_time_ns} ns")
        for scope, cores in sorted((res.per_core_scope_times or {}).items()):
            print("scope", scope, cores)
        if res.instructions_and_trace:
            print("trace path:", res.instructions_and_trace[1])

    out = np.empty((B, S, E), np.float32)
    for c in range(8):
        b, h = c // 2, c % 2
        out[b, rows[h]] = res.results[c]["out"]
    return out



# revision 15
# speedup vs baseline: 1.2265x; 1.2265x over previous
"""Multi-head causal self-attention (B=4, S=2048, E=1024, H=16, D=64) on 8 TRN2 cores.

Sharding: (batch, head-half) tensor parallel — core 2b+h computes batch b,
heads h*8..h*8+7 for ALL 2048 query rows, plus the partial output projection
ctx_half @ Wp[h*512:(h+1)*512].  The host sums the two partial projections of
each batch pair (the "all-reduce").  No duplicated K/V work and no on-device
collectives.

Per-core layout: scores are [kv=partition, q=free] tiles; 8 q-spans of 256
rows with exact causal kv bounds (kv blocks 0..2s+1 for span s).  Score
matmuls for the two heads of a group run CONCURRENTLY in the PE array (row
halves 0-63 / 64-127 via tile_position).  exp on ACT is batched over kv-block
PAIRS ([128, 2 heads, 2 blocks, 256] = 1024 free elems) to amortize the
~352-cycle ACT instruction overhead.  AV matmuls append a ones-row to V
(M=65) so softmax denominators fall out of the same accumulation.
"""

import os
import numpy as np

import concourse.mybir as mybir
import concourse.tile as tile
import concourse.bacc as bacc
from concourse.bass_utils import run_bass_kernel_spmd

B, S, E, H, D = 4, 2048, 1024, 16, 64
HL = 8                   # heads per core (half of H)
EH = HL * D              # 512: per-core head-dim total
KB = S // 128            # 16 kv blocks of 128
NSPAN = 8                # q spans of 256
F32 = mybir.dt.float32
F16 = mybir.dt.float16
SCALE = 1.0 / np.sqrt(D)


def _build_nc():
    nc = bacc.Bacc("TRN2", target_bir_lowering=False, debug=False, num_devices=8)
    xt = nc.dram_tensor("xt", [E, S], F16, kind="ExternalInput")
    wqt = nc.dram_tensor("wqt", [E, EH], F16, kind="ExternalInput")
    wkt = nc.dram_tensor("wkt", [E, EH], F16, kind="ExternalInput")
    wvt = nc.dram_tensor("wvt", [E, EH], F16, kind="ExternalInput")
    wp = nc.dram_tensor("wp", [EH, E], F16, kind="ExternalInput")
    bpv = nc.dram_tensor("bpv", [1, E], F32, kind="ExternalInput")
    masks = nc.dram_tensor("masks", [128, 2, 256], F32, kind="ExternalInput")
    out = nc.dram_tensor("out", [S, E], F16, kind="ExternalOutput")

    xt_r = xt.rearrange("(c p) s -> p c s", p=128)
    wqt_r = wqt.rearrange("(c p) n -> p c n", p=128)
    wkt_r = wkt.rearrange("(c p) n -> p c n", p=128)
    wvt_r = wvt.rearrange("(c p) n -> p c n", p=128)
    wp_r = wp.rearrange("(c p) n -> p c n", p=128)

    with tile.TileContext(nc) as tc:
        with tc.tile_pool(name="persist", bufs=1) as pers, \
             tc.tile_pool(name="ph1", bufs=2) as ph1, \
             tc.tile_pool(name="ph2", bufs=3) as ph2, \
             tc.tile_pool(name="ph2c", bufs=4) as ph2c, \
             tc.tile_pool(name="ph3", bufs=2) as ph3, \
             tc.tile_pool(name="psQ", bufs=2, space="PSUM") as psQ, \
             tc.tile_pool(name="psS", bufs=2, space="PSUM") as psS, \
             tc.tile_pool(name="psC", bufs=1, space="PSUM") as psC, \
             tc.tile_pool(name="dram", bufs=4, space="DRAM") as dram:

            # persistent tiles: K/Q per head-pair (hp) per seq chunk (c);
            # rows 0-63 = head 2hp, 64-127 = head 2hp+1.
            KT = [[pers.tile([128, 512], F16, tag=f"kt{i}_{c}", name=f"kt{i}_{c}")
                   for c in range(4)] for i in range(4)]
            QT = [[pers.tile([128, 512], F16, tag=f"qt{i}_{c}", name=f"qt{i}_{c}")
                   for c in range(4)] for i in range(4)]
            VA = [pers.tile([128, 4, HL, 65], F16, tag=f"va{c}", name=f"va{c}")
                  for c in range(4)]
            CN = [[pers.tile([128, 256], F16, tag=f"cn{i}_{s}", name=f"cn{i}_{s}")
                   for s in range(NSPAN)] for i in range(4)]

            for c in range(4):
                nc.gpsimd.memset(VA[c][:, :, :, 64:65], 1.0)

            mk = pers.tile([128, 2, 256], F32)
            nc.sync.dma_start(mk[:], masks[:])
            wpt = pers.tile([128, 4, E], F16)
            bpb = pers.tile([128, E], F32)

            def emit_wp():
                # deferred: not needed until the first proj, keep it off the
                # startup DMA critical path.
                nc.gpsimd.dma_start(wpt[:], wp_r[:])
                nc.sync.dma_start(bpb[:], bpv[0:1, :].to_broadcast((128, E)))

            def emit_qkv(c):
                """Q, K, V for seq chunk c (512 rows), all 8 local heads."""
                xts = ph1.tile([128, 8, 512], F16, tag="xts")
                nc.gpsimd.dma_start(xts[:], xt_r[:, :, c * 512:(c + 1) * 512])
                for hp in range(4):
                    wq_t = ph1.tile([128, 8, 128], F16, tag="wk")
                    nc.gpsimd.dma_start(wq_t[:], wqt_r[:, :, hp * 128:(hp + 1) * 128])
                    qps = psQ.tile([128, 512], F32, tag="pps")
                    for ec in range(8):
                        nc.tensor.matmul(qps[:], wq_t[:, ec, :], xts[:, ec, :],
                                         start=(ec == 0), stop=(ec == 7))
                    nc.vector.tensor_copy(QT[hp][c][:], qps[:])
                for hp in range(4):
                    wk_t = ph1.tile([128, 8, 128], F16, tag="wk")
                    nc.gpsimd.dma_start(wk_t[:], wkt_r[:, :, hp * 128:(hp + 1) * 128])
                    kps = psQ.tile([128, 512], F32, tag="pps")
                    for ec in range(8):
                        nc.tensor.matmul(kps[:], wk_t[:, ec, :], xts[:, ec, :],
                                         start=(ec == 0), stop=(ec == 7))
                    nc.vector.tensor_copy(KT[hp][c][:], kps[:])
                wv_t = ph1.tile([128, 8, 512], F16, tag="wv")
                nc.gpsimd.dma_start(wv_t[:], wvt_r[:])
                for j in range(4):
                    vps = psQ.tile([128, 512], F32, tag="pps")
                    for ec in range(8):
                        nc.tensor.matmul(vps[:], xts[:, ec, j * 128:(j + 1) * 128],
                                         wv_t[:, ec, :],
                                         start=(ec == 0), stop=(ec == 7))
                    for hl in range(HL):
                        nc.vector.tensor_copy(VA[c][:, j, hl, 0:64],
                                              vps[:, hl * 64:hl * 64 + 64])

            def emit_span(s):
                """Causal attention for q rows 256s..256s+256, kv blocks 0..2s+1."""
                qc, qo = s // 2, (s % 2) * 256
                for hp in range(4):
                    # separate tiles: start=True clears has_written for the
                    # WHOLE bank, so the two heads' accumulators must not share
                    # a PSUM bank.
                    cpsA = psC.tile([65, 256], F32, tag="cpsA")
                    cpsB = psC.tile([65, 256], F32, tag="cpsB")
                    cps = [cpsA, cpsB]

                    def emit_av(pt, pb):
                        for b in range(2):
                            kb = 2 * pb + b
                            c4 = kb // 4
                            st = (pb == 0 and b == 0)
                            en = (pb == s and b == 1)
                            for i in range(2):
                                nc.tensor.matmul(cps[i][:],
                                                 VA[c4][:, kb % 4, 2 * hp + i, :],
                                                 pt[:, i, b, :],
                                                 start=st, stop=en)

                    prev = None
                    for pb in range(s + 1):
                        # scores for kv-block pair (2pb, 2pb+1), both heads.
                        # layout [kv, head, block, q]: head i -> PSUM bank i.
                        sps = psS.tile([128, 2, 2, 256], F32, tag="sps")
                        for b in range(2):
                            kb = 2 * pb + b
                            c4, k0 = kb // 4, (kb % 4) * 128
                            for i in range(2):
                                r0 = i * 64
                                nc.tensor.matmul(
                                    sps[:, i, b, :],
                                    KT[hp][c4][r0:r0 + 64, k0:k0 + 128],
                                    QT[hp][qc][r0:r0 + 64, qo:qo + 256],
                                    start=True, stop=True,
                                    tile_position=(64, 0) if i else None)
                        if pb == s:
                            mka = mk[:].rearrange("p (o b) q -> p o b q", o=1) \
                                .to_broadcast((128, 2, 2, 256))
                            nc.vector.tensor_add(sps[:], sps[:], mka)
                        pt = ph2.tile([128, 2, 2, 256], F16, tag="pt")
                        nc.scalar.activation(pt[:], sps[:],
                                             mybir.ActivationFunctionType.Exp,
                                             scale=float(SCALE))
                        if prev is not None:
                            emit_av(*prev)
                        prev = (pt, pb)
                    emit_av(*prev)
                    # drain PSUM, then normalize by the ones-row sums.
                    ctxu = ph2c.tile([65, 2, 256], F32, tag="ctxu")
                    nc.vector.tensor_copy(ctxu[:, 0, :], cps[0][:])
                    nc.vector.tensor_copy(ctxu[:, 1, :], cps[1][:])
                    rs = ph2c.tile([1, 2, 256], F32, tag="rs")
                    nc.vector.tensor_copy(rs[:], ctxu[64:65, :, :])
                    rr = ph2c.tile([1, 2, 256], F32, tag="rr")
                    nc.vector.reciprocal_approx_fast(rr[:], rs[:])
                    rd = dram.tile([1, 2, 256], F32, tag="rd")
                    nc.sync.dma_start(rd[:], rr[:])
                    bct = ph2c.tile([64, 2, 256], F32, tag="bct")
                    nc.sync.dma_start(bct[:], rd[0:1, :, :].to_broadcast((64, 2, 256)))
                    for i in range(2):
                        nc.vector.tensor_mul(CN[hp][s][i * 64:i * 64 + 64, :],
                                             ctxu[0:64, i, :], bct[:, i, :])

            def emit_proj(s):
                for qq in range(2):
                    q0 = s * 256 + qq * 128
                    for eo in range(2):
                        ops = psQ.tile([128, 512], F32, tag="pps")
                        for hp in range(4):
                            nc.tensor.matmul(ops[:],
                                             CN[hp][s][:, qq * 128:qq * 128 + 128],
                                             wpt[:, hp, eo * 512:(eo + 1) * 512],
                                             start=(hp == 0), stop=(hp == 3))
                        ot = ph3.tile([128, 512], F16, tag="ot")
                        nc.vector.tensor_add(ot[:], ops[:], bpb[:, eo * 512:(eo + 1) * 512])
                        nc.sync.dma_start(out[q0:q0 + 128, eo * 512:(eo + 1) * 512], ot[:])

            emit_qkv(0)
            emit_wp()
            emit_span(0)
            emit_proj(0)
            emit_span(1)
            emit_proj(1)
            emit_qkv(1)
            emit_span(2)
            emit_proj(2)
            emit_span(3)
            emit_proj(3)
            emit_qkv(2)
            emit_span(4)
            emit_proj(4)
            emit_span(5)
            emit_proj(5)
            emit_qkv(3)
            emit_span(6)
            emit_proj(6)
            emit_span(7)
            emit_proj(7)
    nc.compile()
    return nc


_NC_CACHE = None


def _host_masks():
    """Static additive causal masks for the last kv-block pair of any span.

    mask[r, 0, j]: kv block 2s vs q cols (tri for j<128, visible for j>=128)
    mask[r, 1, j]: kv block 2s+1 (fully masked j<128, tri for j>=128)
    """
    r = np.arange(128)[:, None]
    j = np.arange(256)[None, :]
    mask = np.zeros((128, 2, 256), np.float32)
    mask[:, 0, :] = np.where(j >= r, 0.0, -240.0)
    mask[:, 1, :] = np.where((j - 128) >= r, 0.0, -240.0)
    return mask


def kernel(x, Wq, Wk, Wv, Wp, bp):
    global _NC_CACHE
    x = np.asarray(x, np.float32)
    Wq = np.asarray(Wq, np.float32)
    Wk = np.asarray(Wk, np.float32)
    Wv = np.asarray(Wv, np.float32)
    Wp = np.asarray(Wp, np.float32)
    bp = np.asarray(bp, np.float32)

    if _NC_CACHE is None:
        _NC_CACHE = _build_nc()
    nc = _NC_CACHE

    wqt = Wq.transpose(1, 0, 2).reshape(E, E).astype(np.float16)
    wkt = Wk.transpose(1, 0, 2).reshape(E, E).astype(np.float16)
    wvt = Wv.transpose(1, 0, 2).reshape(E, E).astype(np.float16)
    wp16 = Wp.astype(np.float16)
    mask = _host_masks()
    zeros_bp = np.zeros((1, E), np.float32)

    in_maps = []
    for c in range(8):
        b, h = c // 2, c % 2
        cols = slice(h * EH, (h + 1) * EH)
        in_maps.append({
            "xt": np.ascontiguousarray(x[b].T.astype(np.float16)),
            "wqt": np.ascontiguousarray(wqt[:, cols]),
            "wkt": np.ascontiguousarray(wkt[:, cols]),
            "wvt": np.ascontiguousarray(wvt[:, cols]),
            "wp": np.ascontiguousarray(wp16[cols, :]),
            "bpv": bp.reshape(1, E) if h == 0 else zeros_bp,
            "masks": mask,
        })

    trace = bool(os.environ.get("BASS_ATTN_TRACE"))
    res = run_bass_kernel_spmd(nc, in_maps, core_ids=list(range(8)), trace=trace)
    if trace and res.exec_time_ns is not None:
        print(f"HW exec time: {res.exec_time_ns} ns")
        for scope, cores in sorted((res.per_core_scope_times or {}).items()):
            print("scope", scope, cores)
        if res.instructions_and_trace:
            print("trace path:", res.instructions_and_trace[1])

    out = np.empty((B, S, E), np.float32)
    for b in range(B):
        out[b] = res.results[2 * b]["out"].astype(np.float32)
        out[b] += res.results[2 * b + 1]["out"].astype(np.float32)
    return out


# revision 20
# speedup vs baseline: 1.3770x; 1.1227x over previous
"""Multi-head causal self-attention (B=4, S=2048, E=1024, H=16, D=64) on 8 TRN2 cores.

Sharding: (batch, head-half) tensor parallel — core 2b+h computes batch b,
heads h*8..h*8+7 for ALL 2048 query rows, plus the partial output projection
ctx_half @ Wp[h*512:(h+1)*512].  The host sums the two partial projections of
each batch pair (the "all-reduce").  No duplicated K/V work and no on-device
collectives.

Per-core layout: scores are [kv=partition, q=free] tiles; 8 q-spans of 256
rows with exact causal kv bounds (kv blocks 0..2s+1 for span s).  Score
matmuls for the two heads of a group run CONCURRENTLY in the PE array (row
halves 0-63 / 64-127 via tile_position).  exp on ACT is batched over kv-block
PAIRS ([128, 2 heads, 2 blocks, 256] = 1024 free elems) to amortize the
~352-cycle ACT instruction overhead.  AV matmuls append a ones-row to V
(M=65) so softmax denominators fall out of the same accumulation.
"""

import os
import numpy as np

import concourse.mybir as mybir
import concourse.tile as tile
import concourse.bacc as bacc
from concourse.bass_utils import run_bass_kernel_spmd

B, S, E, H, D = 4, 2048, 1024, 16, 64
HL = 8                   # heads per core (half of H)
EH = HL * D              # 512: per-core head-dim total
KB = S // 128            # 16 kv blocks of 128
NSPAN = 4                # q spans of 512
F32 = mybir.dt.float32
F16 = mybir.dt.float16
SCALE = 1.0 / np.sqrt(D)


def _build_nc():
    nc = bacc.Bacc("TRN2", target_bir_lowering=False, debug=False, num_devices=8)
    xt = nc.dram_tensor("xt", [E, S], F16, kind="ExternalInput")
    wqt = nc.dram_tensor("wqt", [E, EH], F16, kind="ExternalInput")
    wkt = nc.dram_tensor("wkt", [E, EH], F16, kind="ExternalInput")
    wvt = nc.dram_tensor("wvt", [E, EH], F16, kind="ExternalInput")
    wp = nc.dram_tensor("wp", [EH, E], F16, kind="ExternalInput")
    bpv = nc.dram_tensor("bpv", [1, E], F32, kind="ExternalInput")
    masks = nc.dram_tensor("masks", [128, 128], F32, kind="ExternalInput")
    out = nc.dram_tensor("out", [S, E], F16, kind="ExternalOutput")

    xt_r = xt.rearrange("(c p) s -> p c s", p=128)
    wqt_r = wqt.rearrange("(c p) n -> p c n", p=128)
    wkt_r = wkt.rearrange("(c p) n -> p c n", p=128)
    wvt_r = wvt.rearrange("(c p) n -> p c n", p=128)
    wp_r = wp.rearrange("(c p) n -> p c n", p=128)

    with tile.TileContext(nc) as tc:
        with tc.tile_pool(name="persist", bufs=1) as pers, \
             tc.tile_pool(name="ph1", bufs=2) as ph1, \
             tc.tile_pool(name="ph2", bufs=3) as ph2, \
             tc.tile_pool(name="ph2c", bufs=4) as ph2c, \
             tc.tile_pool(name="ph3", bufs=2) as ph3, \
             tc.tile_pool(name="psQ", bufs=2, space="PSUM") as psQ, \
             tc.tile_pool(name="psS", bufs=2, space="PSUM") as psS, \
             tc.tile_pool(name="psC", bufs=1, space="PSUM") as psC, \
             tc.tile_pool(name="dram", bufs=4, space="DRAM") as dram:

            # persistent tiles: K/Q per head-pair (hp) per seq chunk (c);
            # rows 0-63 = head 2hp, 64-127 = head 2hp+1.
            KT = [[pers.tile([128, 512], F16, tag=f"kt{i}_{c}", name=f"kt{i}_{c}")
                   for c in range(4)] for i in range(4)]
            QT = [[pers.tile([128, 512], F16, tag=f"qt{i}_{c}", name=f"qt{i}_{c}")
                   for c in range(4)] for i in range(4)]
            VA = [pers.tile([128, 4, HL, 65], F16, tag=f"va{c}", name=f"va{c}")
                  for c in range(4)]
            CN = [[pers.tile([128, 512], F16, tag=f"cn{i}_{s}", name=f"cn{i}_{s}")
                   for s in range(NSPAN)] for i in range(4)]

            for c in range(4):
                nc.gpsimd.memset(VA[c][:, :, :, 64:65], 1.0)

            mk = pers.tile([128, 128], F32)
            nc.sync.dma_start(mk[:], masks[:])
            wpt = pers.tile([128, 4, E], F16)
            bpb = pers.tile([128, E], F32)

            def emit_wp():
                # deferred: not needed until the first proj, keep it off the
                # startup DMA critical path.
                nc.gpsimd.dma_start(wpt[:], wp_r[:])
                nc.sync.dma_start(bpb[:], bpv[0:1, :].to_broadcast((128, E)))

            def emit_qkv(c):
                """Q, K, V for seq chunk c (512 rows), all 8 local heads."""
                xts = ph1.tile([128, 8, 512], F16, tag="xts")
                nc.gpsimd.dma_start(xts[:], xt_r[:, :, c * 512:(c + 1) * 512])
                for hp in range(4):
                    wq_t = ph1.tile([128, 8, 128], F16, tag="wk")
                    nc.gpsimd.dma_start(wq_t[:], wqt_r[:, :, hp * 128:(hp + 1) * 128])
                    qps = psQ.tile([128, 512], F32, tag="pps")
                    for ec in range(8):
                        nc.tensor.matmul(qps[:], wq_t[:, ec, :], xts[:, ec, :],
                                         start=(ec == 0), stop=(ec == 7))
                    nc.vector.tensor_copy(QT[hp][c][:], qps[:])
                for hp in range(4):
                    wk_t = ph1.tile([128, 8, 128], F16, tag="wk")
                    nc.gpsimd.dma_start(wk_t[:], wkt_r[:, :, hp * 128:(hp + 1) * 128])
                    kps = psQ.tile([128, 512], F32, tag="pps")
                    for ec in range(8):
                        nc.tensor.matmul(kps[:], wk_t[:, ec, :], xts[:, ec, :],
                                         start=(ec == 0), stop=(ec == 7))
                    nc.vector.tensor_copy(KT[hp][c][:], kps[:])
                wv_t = ph1.tile([128, 8, 512], F16, tag="wv")
                nc.gpsimd.dma_start(wv_t[:], wvt_r[:])
                for j in range(4):
                    vps = psQ.tile([128, 512], F32, tag="pps")
                    for ec in range(8):
                        nc.tensor.matmul(vps[:], xts[:, ec, j * 128:(j + 1) * 128],
                                         wv_t[:, ec, :],
                                         start=(ec == 0), stop=(ec == 7))
                    nc.vector.tensor_copy(
                        VA[c][:, j, :, 0:64],
                        vps[:].rearrange("p (h d) -> p h d", h=HL))

            def emit_span(s, fillers=()):
                """Causal attention for q rows 512s..512s+512, kv blocks 0..4s+3.

                Diagonal kv block 4s+b only sees q cols >= 128b: score/exp/AV
                are sliced to that region, so no flat -inf mask is ever needed
                (just the [128,128] triangle on the diagonal sub-block).
                fillers: callables (proj pieces) emitted between hp groups to
                keep TensorE fed while ACT grinds this span's exps.
                """
                fillers = list(fillers)
                for hp in range(4):
                    # separate tiles: start=True clears has_written for the
                    # WHOLE bank, so the two heads' accumulators must not share
                    # a PSUM bank.
                    cpsA = psC.tile([65, 512], F32, tag="cpsA")
                    cpsB = psC.tile([65, 512], F32, tag="cpsB")
                    cps = [cpsA, cpsB]
                    last = 4 * s + 3

                    def emit_av(pt, kb, lo):
                        c4 = kb // 4
                        for i in range(2):
                            nc.tensor.matmul(cps[i][:, lo:512],
                                             VA[c4][:, kb % 4, 2 * hp + i, :],
                                             pt[:, i, lo:512],
                                             start=(kb == 0), stop=(kb == last))

                    prev = None
                    for kb in range(last + 1):
                        b = kb - 4 * s
                        lo = 128 * b if b > 0 else 0
                        c4, k0 = kb // 4, (kb % 4) * 128
                        # scores [kv, head, q]: head i -> own PSUM bank, the
                        # two heads' matmuls run concurrently in row halves.
                        sps = psS.tile([128, 2, 512], F32, tag="sps")
                        for i in range(2):
                            r0 = i * 64
                            nc.tensor.matmul(
                                sps[:, i, lo:512],
                                KT[hp][c4][r0:r0 + 64, k0:k0 + 128],
                                QT[hp][s][r0:r0 + 64, lo:512],
                                start=True, stop=True,
                                tile_position=(64, 0) if i else None)
                        if b >= 0:
                            mka = mk[:].rearrange("p (o j) -> p o j", o=1) \
                                .to_broadcast((128, 2, 128))
                            nc.vector.tensor_add(sps[:, :, lo:lo + 128],
                                                 sps[:, :, lo:lo + 128], mka)
                        pt = ph2.tile([128, 2, 512], F16, tag="pt")
                        nc.scalar.activation(pt[:, :, lo:512], sps[:, :, lo:512],
                                             mybir.ActivationFunctionType.Exp,
                                             scale=float(SCALE))
                        if prev is not None:
                            emit_av(*prev)
                        prev = (pt, kb, lo)
                    emit_av(*prev)
                    # drain PSUM, then normalize by the ones-row sums.
                    ctxu = ph2c.tile([65, 2, 512], F32, tag="ctxu")
                    nc.vector.tensor_copy(ctxu[:, 0, :], cps[0][:])
                    nc.vector.tensor_copy(ctxu[:, 1, :], cps[1][:])
                    rs = ph2c.tile([1, 2, 512], F32, tag="rs")
                    nc.vector.tensor_copy(rs[:], ctxu[64:65, :, :])
                    rr = ph2c.tile([1, 2, 512], F32, tag="rr")
                    nc.vector.reciprocal_approx_fast(rr[:], rs[:])
                    rd = dram.tile([1, 2, 512], F32, tag="rd")
                    nc.sync.dma_start(rd[:], rr[:])
                    bct = ph2c.tile([64, 2, 512], F32, tag="bct")
                    nc.sync.dma_start(bct[:], rd[0:1, :, :].to_broadcast((64, 2, 512)))
                    for i in range(2):
                        nc.vector.tensor_mul(CN[hp][s][i * 64:i * 64 + 64, :],
                                             ctxu[0:64, i, :], bct[:, i, :])
                    # feed TensorE between hp groups (proj pieces of span s-1)
                    for _ in range(2):
                        if fillers:
                            fillers.pop(0)()
                for f in fillers:
                    f()

            def proj_pieces(s):
                def piece(qq, eo):
                    def run():
                        q0 = s * 512 + qq * 128
                        ops = psQ.tile([128, 512], F32, tag="pps")
                        for hp in range(4):
                            nc.tensor.matmul(ops[:],
                                             CN[hp][s][:, qq * 128:qq * 128 + 128],
                                             wpt[:, hp, eo * 512:(eo + 1) * 512],
                                             start=(hp == 0), stop=(hp == 3))
                        ot = ph3.tile([128, 512], F16, tag="ot")
                        nc.vector.tensor_add(ot[:], ops[:],
                                             bpb[:, eo * 512:(eo + 1) * 512])
                        nc.sync.dma_start(out[q0:q0 + 128,
                                              eo * 512:(eo + 1) * 512], ot[:])
                    return run
                return [piece(qq, eo) for qq in range(4) for eo in range(2)]

            emit_qkv(0)
            emit_wp()
            emit_span(0)
            emit_qkv(1)
            emit_span(1, proj_pieces(0))
            emit_qkv(2)
            emit_span(2, proj_pieces(1))
            emit_qkv(3)
            emit_span(3, proj_pieces(2))
            for f in proj_pieces(3):
                f()
    nc.compile()
    return nc


_NC_CACHE = None


def _host_masks():
    """Static additive causal triangle for any diagonal 128x128 sub-block."""
    r = np.arange(128)[:, None]
    j = np.arange(128)[None, :]
    return np.where(j >= r, 0.0, -240.0).astype(np.float32)


def kernel(x, Wq, Wk, Wv, Wp, bp):
    global _NC_CACHE
    x = np.asarray(x, np.float32)
    Wq = np.asarray(Wq, np.float32)
    Wk = np.asarray(Wk, np.float32)
    Wv = np.asarray(Wv, np.float32)
    Wp = np.asarray(Wp, np.float32)
    bp = np.asarray(bp, np.float32)

    if _NC_CACHE is None:
        _NC_CACHE = _build_nc()
    nc = _NC_CACHE

    wqt = Wq.transpose(1, 0, 2).reshape(E, E).astype(np.float16)
    wkt = Wk.transpose(1, 0, 2).reshape(E, E).astype(np.float16)
    wvt = Wv.transpose(1, 0, 2).reshape(E, E).astype(np.float16)
    wp16 = Wp.astype(np.float16)
    mask = _host_masks()
    zeros_bp = np.zeros((1, E), np.float32)

    in_maps = []
    for c in range(8):
        b, h = c // 2, c % 2
        cols = slice(h * EH, (h + 1) * EH)
        in_maps.append({
            "xt": np.ascontiguousarray(x[b].T.astype(np.float16)),
            "wqt": np.ascontiguousarray(wqt[:, cols]),
            "wkt": np.ascontiguousarray(wkt[:, cols]),
            "wvt": np.ascontiguousarray(wvt[:, cols]),
            "wp": np.ascontiguousarray(wp16[cols, :]),
            "bpv": bp.reshape(1, E) if h == 0 else zeros_bp,
            "masks": mask,
        })

    trace = bool(os.environ.get("BASS_ATTN_TRACE"))
    res = run_bass_kernel_spmd(nc, in_maps, core_ids=list(range(8)), trace=trace)
    if trace and res.exec_time_ns is not None:
        print(f"HW exec time: {res.exec_time_ns} ns")
        for scope, cores in sorted((res.per_core_scope_times or {}).items()):
            print("scope", scope, cores)
        if res.instructions_and_trace:
            print("trace path:", res.instructions_and_trace[1])

    out = np.empty((B, S, E), np.float32)
    for b in range(B):
        out[b] = res.results[2 * b]["out"].astype(np.float32)
        out[b] += res.results[2 * b + 1]["out"].astype(np.float32)
    return out


# revision 31
# speedup vs baseline: 1.5797x; 1.1472x over previous
"""Multi-head causal self-attention (B=4, S=2048, E=1024, H=16, D=64) on 8 TRN2 cores.

Sharding: (batch, head-half) tensor parallel — core 2b+h computes batch b,
heads h*8..h*8+7 for ALL 2048 query rows, plus the partial output projection
ctx_half @ Wp[h*512:(h+1)*512].  The host sums the two partial projections of
each batch pair (the "all-reduce").  No duplicated K/V work and no on-device
collectives.

Per-core layout: scores are [kv=partition, q=free] tiles; 8 q-spans of 256
rows with exact causal kv bounds (kv blocks 0..2s+1 for span s).  Score
matmuls for the two heads of a group run CONCURRENTLY in the PE array (row
halves 0-63 / 64-127 via tile_position).  exp on ACT is batched over kv-block
PAIRS ([128, 2 heads, 2 blocks, 256] = 1024 free elems) to amortize the
~352-cycle ACT instruction overhead.  AV matmuls append a ones-row to V
(M=65) so softmax denominators fall out of the same accumulation.
"""

import os
import numpy as np

import concourse.mybir as mybir
import concourse.tile as tile
import concourse.bacc as bacc
from concourse.bass_utils import run_bass_kernel_spmd

B, S, E, H, D = 4, 2048, 1024, 16, 64
HL = 8                   # heads per core (half of H)
EH = HL * D              # 512: per-core head-dim total
KB = S // 128            # 16 kv blocks of 128
NSPAN = 4                # q spans of 512
F32 = mybir.dt.float32
F16 = mybir.dt.float16
SCALE = 1.0 / np.sqrt(D)


def _build_nc():
    nc = bacc.Bacc("TRN2", target_bir_lowering=False, debug=False, num_devices=8)
    xt = nc.dram_tensor("xt", [E, S], F16, kind="ExternalInput")
    wqt = nc.dram_tensor("wqt", [E, EH], F16, kind="ExternalInput")
    wkt = nc.dram_tensor("wkt", [E, EH], F16, kind="ExternalInput")
    wvt = nc.dram_tensor("wvt", [E, EH], F16, kind="ExternalInput")
    wp = nc.dram_tensor("wp", [EH, E], F16, kind="ExternalInput")
    masks = nc.dram_tensor("masks", [128, 128], F32, kind="ExternalInput")
    out = nc.dram_tensor("out", [S, E], F16, kind="ExternalOutput")

    xt_r = xt.rearrange("(c p) s -> p c s", p=128)
    wqt_r = wqt.rearrange("(c p) n -> p c n", p=128)
    wkt_r = wkt.rearrange("(c p) n -> p c n", p=128)
    wvt_r = wvt.rearrange("(c p) n -> p c n", p=128)
    wp_r = wp.rearrange("(c p) n -> p c n", p=128)

    with tile.TileContext(nc) as tc:
        with tc.tile_pool(name="persist", bufs=1) as pers, \
             tc.tile_pool(name="ph1", bufs=2) as ph1, \
             tc.tile_pool(name="ph2", bufs=3) as ph2, \
             tc.tile_pool(name="ph2c", bufs=2) as ph2c, \
             tc.tile_pool(name="ph3", bufs=2) as ph3, \
             tc.tile_pool(name="psQ", bufs=2, space="PSUM") as psQ, \
             tc.tile_pool(name="psS", bufs=2, space="PSUM") as psS, \
             tc.tile_pool(name="psC", bufs=1, space="PSUM") as psC, \
             tc.tile_pool(name="dram", bufs=4, space="DRAM") as dram:

            # persistent tiles: K/Q per head-pair (hp) per seq chunk (c);
            # rows 0-63 = head 2hp, 64-127 = head 2hp+1.
            KT = [[pers.tile([128, 512], F16, tag=f"kt{i}_{c}", name=f"kt{i}_{c}")
                   for c in range(4)] for i in range(4)]
            QT = [[pers.tile([128, 512], F16, tag=f"qt{i}_{c}", name=f"qt{i}_{c}")
                   for c in range(4)] for i in range(4)]
            VA = [pers.tile([128, 4, HL, 65], F16, tag=f"va{c}", name=f"va{c}")
                  for c in range(4)]
            CN = [[pers.tile([128, 512], F16, tag=f"cn{i}_{s}", name=f"cn{i}_{s}")
                   for s in range(NSPAN)] for i in range(4)]

            for c in range(4):
                nc.gpsimd.memset(VA[c][:, :, :, 64:65], 1.0)

            mk = pers.tile([128, 128], F32)
            nc.sync.dma_start(mk[:], masks[:])
            wpt = pers.tile([128, 4, E], F16)

            def emit_wp():
                # deferred: not needed until the first proj, keep it off the
                # startup DMA critical path.
                nc.gpsimd.dma_start(wpt[:], wp_r[:])

            def emit_qkv(c):
                """Q, K, V for seq chunk c (512 rows), all 8 local heads."""
                # first weight DMA ahead of the 1MB xts transfer so the first
                # MM group isn't queued behind it.
                w0 = ph1.tile([128, 8, 128], F16, tag="wk", name=f"w0_{c}")
                nc.gpsimd.dma_start(w0[:], wqt_r[:, :, 0:128])
                xts = ph1.tile([128, 8, 512], F16, tag="xts")
                nc.gpsimd.dma_start(xts[:], xt_r[:, :, c * 512:(c + 1) * 512])
                for hp in range(4):
                    if hp == 0:
                        wq_t = w0
                    else:
                        wq_t = ph1.tile([128, 8, 128], F16, tag="wk",
                                        name=f"wq{hp}_{c}")
                        nc.gpsimd.dma_start(wq_t[:],
                                            wqt_r[:, :, hp * 128:(hp + 1) * 128])
                    qps = psQ.tile([128, 512], F32, tag="pps")
                    for ec in range(8):
                        nc.tensor.matmul(qps[:], wq_t[:, ec, :], xts[:, ec, :],
                                         start=(ec == 0), stop=(ec == 7))
                    nc.vector.tensor_copy(QT[hp][c][:], qps[:])
                for hp in range(4):
                    wk_t = ph1.tile([128, 8, 128], F16, tag="wk",
                                    name=f"wk{hp}_{c}")
                    nc.gpsimd.dma_start(wk_t[:], wkt_r[:, :, hp * 128:(hp + 1) * 128])
                    kps = psQ.tile([128, 512], F32, tag="pps")
                    for ec in range(8):
                        nc.tensor.matmul(kps[:], wk_t[:, ec, :], xts[:, ec, :],
                                         start=(ec == 0), stop=(ec == 7))
                    nc.vector.tensor_copy(KT[hp][c][:], kps[:])
                wv_t = ph1.tile([128, 8, 512], F16, tag="wv")
                nc.gpsimd.dma_start(wv_t[:], wvt_r[:])
                for j in range(4):
                    vps = psQ.tile([128, 512], F32, tag="pps")
                    for ec in range(8):
                        nc.tensor.matmul(vps[:], xts[:, ec, j * 128:(j + 1) * 128],
                                         wv_t[:, ec, :],
                                         start=(ec == 0), stop=(ec == 7))
                    nc.vector.tensor_copy(
                        VA[c][:, j, :, 0:64],
                        vps[:].rearrange("p (h d) -> p h d", h=HL))

            def emit_span(s, fillers=()):
                """Causal attention for q rows 512s..512s+512, kv blocks 0..4s+3.

                Diagonal kv block 4s+b only sees q cols >= 128b: score/exp/AV
                are sliced to that region, so no flat -inf mask is ever needed
                (just the [128,128] triangle on the diagonal sub-block).
                fillers: callables (proj pieces) emitted between hp groups to
                keep TensorE fed while ACT grinds this span's exps.
                """
                fillers = list(fillers)
                for hp in range(4):
                    # separate tiles: start=True clears has_written for the
                    # WHOLE bank, so the two heads' accumulators must not share
                    # a PSUM bank.
                    cpsA = psC.tile([65, 512], F32, tag="cpsA")
                    cpsB = psC.tile([65, 512], F32, tag="cpsB")
                    cps = [cpsA, cpsB]
                    last = 4 * s + 3

                    def emit_av(pt, kb, lo):
                        c4 = kb // 4
                        for i in range(2):
                            nc.tensor.matmul(cps[i][:, lo:512],
                                             VA[c4][:, kb % 4, 2 * hp + i, :],
                                             pt[:, i, lo:512],
                                             start=(kb == 0), stop=(kb == last))

                    prev = None
                    for kb in range(last + 1):
                        b = kb - 4 * s
                        lo = 128 * b if b > 0 else 0
                        c4, k0 = kb // 4, (kb % 4) * 128
                        # scores [kv, head, q]: head i -> own PSUM bank, the
                        # two heads' matmuls run concurrently in row halves.
                        sps = psS.tile([128, 2, 512], F32, tag="sps")
                        for i in range(2):
                            r0 = i * 64
                            nc.tensor.matmul(
                                sps[:, i, lo:512],
                                KT[hp][c4][r0:r0 + 64, k0:k0 + 128],
                                QT[hp][s][r0:r0 + 64, lo:512],
                                start=True, stop=True,
                                tile_position=(64, 0) if i else None)
                        if b >= 0:
                            mka = mk[:].rearrange("p (o j) -> p o j", o=1) \
                                .to_broadcast((128, 2, 128))
                            nc.vector.tensor_add(sps[:, :, lo:lo + 128],
                                                 sps[:, :, lo:lo + 128], mka)
                        pt = ph2.tile([128, 2, 512], F16, tag="pt")
                        nc.scalar.activation(pt[:, :, lo:512], sps[:, :, lo:512],
                                             mybir.ActivationFunctionType.Exp,
                                             scale=float(SCALE))
                        if prev is not None:
                            emit_av(*prev)
                        prev = (pt, kb, lo)
                    emit_av(*prev)
                    # drain PSUM, then normalize by the ones-row sums.
                    ctxu = ph2c.tile([65, 2, 512], F32, tag="ctxu")
                    nc.vector.tensor_copy(ctxu[:, 0, :], cps[0][:])
                    nc.vector.tensor_copy(ctxu[:, 1, :], cps[1][:])
                    rs = ph2c.tile([1, 2, 512], F32, tag="rs")
                    nc.vector.tensor_copy(rs[:], ctxu[64:65, :, :])
                    rr = ph2c.tile([1, 2, 512], F32, tag="rr")
                    nc.vector.reciprocal_approx_fast(rr[:], rs[:])
                    rd = dram.tile([1, 2, 512], F32, tag="rd")
                    nc.gpsimd.dma_start(rd[:], rr[:])
                    bct = ph2c.tile([64, 2, 512], F32, tag="bct")
                    nc.gpsimd.dma_start(bct[:], rd[0:1, :, :].to_broadcast((64, 2, 512)))
                    for i in range(2):
                        nc.vector.tensor_mul(CN[hp][s][i * 64:i * 64 + 64, :],
                                             ctxu[0:64, i, :], bct[:, i, :])
                    # feed TensorE between hp groups (proj pieces of span s-1)
                    for _ in range(2):
                        if fillers:
                            fillers.pop(0)()
                for f in fillers:
                    f()

            def proj_pieces(s):
                def piece(qq, eo):
                    def run():
                        q0 = s * 512 + qq * 128
                        ops = psQ.tile([128, 512], F32, tag="pps")
                        for hp in range(4):
                            nc.tensor.matmul(ops[:],
                                             CN[hp][s][:, qq * 128:qq * 128 + 128],
                                             wpt[:, hp, eo * 512:(eo + 1) * 512],
                                             start=(hp == 0), stop=(hp == 3))
                        # bias is added host-side during the pair reduce.
                        ot = ph3.tile([128, 512], F16, tag="ot")
                        nc.vector.tensor_copy(ot[:], ops[:])
                        nc.sync.dma_start(out[q0:q0 + 128,
                                              eo * 512:(eo + 1) * 512], ot[:])
                    return run
                return [piece(qq, eo) for qq in range(4) for eo in range(2)]

            emit_qkv(0)
            emit_wp()
            emit_span(0)
            emit_qkv(1)
            emit_span(1, proj_pieces(0))
            emit_qkv(2)
            emit_span(2, proj_pieces(1))
            emit_qkv(3)
            emit_span(3, proj_pieces(2))
            for f in proj_pieces(3):
                f()
    nc.compile()
    return nc


_NC_CACHE = None


def _host_masks():
    """Static additive causal triangle for any diagonal 128x128 sub-block."""
    r = np.arange(128)[:, None]
    j = np.arange(128)[None, :]
    return np.where(j >= r, 0.0, -240.0).astype(np.float32)


def kernel(x, Wq, Wk, Wv, Wp, bp):
    global _NC_CACHE
    x = np.asarray(x, np.float32)
    Wq = np.asarray(Wq, np.float32)
    Wk = np.asarray(Wk, np.float32)
    Wv = np.asarray(Wv, np.float32)
    Wp = np.asarray(Wp, np.float32)
    bp = np.asarray(bp, np.float32)

    if _NC_CACHE is None:
        _NC_CACHE = _build_nc()
    nc = _NC_CACHE

    wqt = Wq.transpose(1, 0, 2).reshape(E, E).astype(np.float16)
    wkt = Wk.transpose(1, 0, 2).reshape(E, E).astype(np.float16)
    wvt = Wv.transpose(1, 0, 2).reshape(E, E).astype(np.float16)
    wp16 = Wp.astype(np.float16)
    mask = _host_masks()

    in_maps = []
    for c in range(8):
        b, h = c // 2, c % 2
        cols = slice(h * EH, (h + 1) * EH)
        in_maps.append({
            "xt": np.ascontiguousarray(x[b].T.astype(np.float16)),
            "wqt": np.ascontiguousarray(wqt[:, cols]),
            "wkt": np.ascontiguousarray(wkt[:, cols]),
            "wvt": np.ascontiguousarray(wvt[:, cols]),
            "wp": np.ascontiguousarray(wp16[cols, :]),
            "masks": mask,
        })

    trace = bool(os.environ.get("BASS_ATTN_TRACE"))
    res = run_bass_kernel_spmd(nc, in_maps, core_ids=list(range(8)), trace=trace)
    if trace and res.exec_time_ns is not None:
        print(f"HW exec time: {res.exec_time_ns} ns")
        for scope, cores in sorted((res.per_core_scope_times or {}).items()):
            print("scope", scope, cores)
        if res.instructions_and_trace:
            print("trace path:", res.instructions_and_trace[1])

    out = np.empty((B, S, E), np.float32)
    for b in range(B):
        out[b] = res.results[2 * b]["out"].astype(np.float32)
        out[b] += res.results[2 * b + 1]["out"].astype(np.float32)
        out[b] += bp
    return out


# revision 32
# speedup vs baseline: 1.7364x; 1.0992x over previous
"""Multi-head causal self-attention (B=4, S=2048, E=1024, H=16, D=64) on 8 TRN2 cores.

Sharding: (batch, head-half) tensor parallel — core 2b+h computes batch b,
heads h*8..h*8+7 for ALL 2048 query rows, plus the partial output projection
ctx_half @ Wp[h*512:(h+1)*512].  The host sums the two partial projections of
each batch pair (the "all-reduce").  No duplicated K/V work and no on-device
collectives.

Per-core layout: scores are [kv=partition, q=free] tiles; 8 q-spans of 256
rows with exact causal kv bounds (kv blocks 0..2s+1 for span s).  Score
matmuls for the two heads of a group run CONCURRENTLY in the PE array (row
halves 0-63 / 64-127 via tile_position).  exp on ACT is batched over kv-block
PAIRS ([128, 2 heads, 2 blocks, 256] = 1024 free elems) to amortize the
~352-cycle ACT instruction overhead.  AV matmuls append a ones-row to V
(M=65) so softmax denominators fall out of the same accumulation.
"""

import os
import numpy as np

import concourse.mybir as mybir
import concourse.tile as tile
import concourse.bacc as bacc
from concourse.bass_utils import run_bass_kernel_spmd

B, S, E, H, D = 4, 2048, 1024, 16, 64
HL = 8                   # heads per core (half of H)
EH = HL * D              # 512: per-core head-dim total
KB = S // 128            # 16 kv blocks of 128
NSPAN = 4                # q spans of 512
F32 = mybir.dt.float32
F16 = mybir.dt.float16
SCALE = 1.0 / np.sqrt(D)


def _build_nc():
    nc = bacc.Bacc("TRN2", target_bir_lowering=False, debug=False, num_devices=8)
    xt = nc.dram_tensor("xt", [E, S], F16, kind="ExternalInput")
    wqt = nc.dram_tensor("wqt", [E, EH], F16, kind="ExternalInput")
    wkt = nc.dram_tensor("wkt", [E, EH], F16, kind="ExternalInput")
    wvt = nc.dram_tensor("wvt", [E, EH], F16, kind="ExternalInput")
    wp = nc.dram_tensor("wp", [EH, E], F16, kind="ExternalInput")
    masks = nc.dram_tensor("masks", [128, 128], F32, kind="ExternalInput")
    out = nc.dram_tensor("out", [S, E], F16, kind="ExternalOutput")

    xt_r = xt.rearrange("(c p) s -> p c s", p=128)
    wqt_r = wqt.rearrange("(c p) n -> p c n", p=128)
    wkt_r = wkt.rearrange("(c p) n -> p c n", p=128)
    wvt_r = wvt.rearrange("(c p) n -> p c n", p=128)
    wp_r = wp.rearrange("(c p) n -> p c n", p=128)

    with tile.TileContext(nc) as tc:
        with tc.tile_pool(name="persist", bufs=1) as pers, \
             tc.tile_pool(name="ph1", bufs=2) as ph1, \
             tc.tile_pool(name="ph2", bufs=3) as ph2, \
             tc.tile_pool(name="ph2c", bufs=2) as ph2c, \
             tc.tile_pool(name="ph3", bufs=2) as ph3, \
             tc.tile_pool(name="psQ", bufs=2, space="PSUM") as psQ, \
             tc.tile_pool(name="psS", bufs=2, space="PSUM") as psS, \
             tc.tile_pool(name="psC", bufs=1, space="PSUM") as psC, \
             tc.tile_pool(name="dram", bufs=4, space="DRAM") as dram:

            # persistent tiles: K/Q per head-pair (hp) per seq chunk (c);
            # rows 0-63 = head 2hp, 64-127 = head 2hp+1.
            KT = [[pers.tile([128, 512], F16, tag=f"kt{i}_{c}", name=f"kt{i}_{c}")
                   for c in range(4)] for i in range(4)]
            QT = [[pers.tile([128, 512], F16, tag=f"qt{i}_{c}", name=f"qt{i}_{c}")
                   for c in range(4)] for i in range(4)]
            VA = [pers.tile([128, 4, HL, 65], F16, tag=f"va{c}", name=f"va{c}")
                  for c in range(4)]
            CN = [[pers.tile([128, 512], F16, tag=f"cn{i}_{s}", name=f"cn{i}_{s}")
                   for s in range(NSPAN)] for i in range(4)]

            for c in range(4):
                nc.gpsimd.memset(VA[c][:, :, :, 64:65], 1.0)

            mk = pers.tile([128, 128], F32)
            nc.sync.dma_start(mk[:], masks[:])
            wpt = pers.tile([128, 4, E], F16)

            # x and all weights live in SBUF for the whole kernel, DMA'd once
            # up front in slices so the first matmuls start after ~3.5us and
            # there are no pool-rotation stalls at chunk boundaries.
            XA = pers.tile([128, 8, S], F16)
            WQ = pers.tile([128, 8, EH], F16)
            WK = pers.tile([128, 8, EH], F16)
            WV = pers.tile([128, 8, EH], F16)
            nc.gpsimd.dma_start(XA[:, :, 0:512], xt_r[:, :, 0:512])
            for hp in range(4):
                nc.gpsimd.dma_start(WQ[:, :, hp * 128:(hp + 1) * 128],
                                    wqt_r[:, :, hp * 128:(hp + 1) * 128])
            for hp in range(4):
                nc.gpsimd.dma_start(WK[:, :, hp * 128:(hp + 1) * 128],
                                    wkt_r[:, :, hp * 128:(hp + 1) * 128])
            nc.gpsimd.dma_start(WV[:], wvt_r[:])
            for c in range(1, 4):
                nc.gpsimd.dma_start(XA[:, :, c * 512:(c + 1) * 512],
                                    xt_r[:, :, c * 512:(c + 1) * 512])

            def emit_wp():
                # deferred: not needed until the first proj, keep it off the
                # startup DMA critical path.
                nc.gpsimd.dma_start(wpt[:], wp_r[:])

            def emit_qkv(c):
                """Q, K, V for seq chunk c (512 rows), all 8 local heads."""
                cs = slice(c * 512, (c + 1) * 512)
                for hp in range(4):
                    qps = psQ.tile([128, 512], F32, tag="pps")
                    for ec in range(8):
                        nc.tensor.matmul(qps[:],
                                         WQ[:, ec, hp * 128:(hp + 1) * 128],
                                         XA[:, ec, cs],
                                         start=(ec == 0), stop=(ec == 7))
                    nc.vector.tensor_copy(QT[hp][c][:], qps[:])
                for hp in range(4):
                    kps = psQ.tile([128, 512], F32, tag="pps")
                    for ec in range(8):
                        nc.tensor.matmul(kps[:],
                                         WK[:, ec, hp * 128:(hp + 1) * 128],
                                         XA[:, ec, cs],
                                         start=(ec == 0), stop=(ec == 7))
                    nc.vector.tensor_copy(KT[hp][c][:], kps[:])
                for j in range(4):
                    vps = psQ.tile([128, 512], F32, tag="pps")
                    for ec in range(8):
                        nc.tensor.matmul(vps[:],
                                         XA[:, ec, c * 512 + j * 128:
                                            c * 512 + (j + 1) * 128],
                                         WV[:, ec, :],
                                         start=(ec == 0), stop=(ec == 7))
                    nc.vector.tensor_copy(
                        VA[c][:, j, :, 0:64],
                        vps[:].rearrange("p (h d) -> p h d", h=HL))

            def emit_span(s, fillers=()):
                """Causal attention for q rows 512s..512s+512, kv blocks 0..4s+3.

                Diagonal kv block 4s+b only sees q cols >= 128b: score/exp/AV
                are sliced to that region, so no flat -inf mask is ever needed
                (just the [128,128] triangle on the diagonal sub-block).
                fillers: callables (proj pieces) emitted between hp groups to
                keep TensorE fed while ACT grinds this span's exps.
                """
                fillers = list(fillers)
                for hp in range(4):
                    # separate tiles: start=True clears has_written for the
                    # WHOLE bank, so the two heads' accumulators must not share
                    # a PSUM bank.
                    cpsA = psC.tile([65, 512], F32, tag="cpsA")
                    cpsB = psC.tile([65, 512], F32, tag="cpsB")
                    cps = [cpsA, cpsB]
                    last = 4 * s + 3

                    def emit_av(pt, kb, lo):
                        c4 = kb // 4
                        for i in range(2):
                            nc.tensor.matmul(cps[i][:, lo:512],
                                             VA[c4][:, kb % 4, 2 * hp + i, :],
                                             pt[:, i, lo:512],
                                             start=(kb == 0), stop=(kb == last))

                    prev = None
                    for kb in range(last + 1):
                        b = kb - 4 * s
                        lo = 128 * b if b > 0 else 0
                        c4, k0 = kb // 4, (kb % 4) * 128
                        # scores [kv, head, q]: head i -> own PSUM bank, the
                        # two heads' matmuls run concurrently in row halves.
                        sps = psS.tile([128, 2, 512], F32, tag="sps")
                        for i in range(2):
                            r0 = i * 64
                            nc.tensor.matmul(
                                sps[:, i, lo:512],
                                KT[hp][c4][r0:r0 + 64, k0:k0 + 128],
                                QT[hp][s][r0:r0 + 64, lo:512],
                                start=True, stop=True,
                                tile_position=(64, 0) if i else None)
                        if b >= 0:
                            mka = mk[:].rearrange("p (o j) -> p o j", o=1) \
                                .to_broadcast((128, 2, 128))
                            nc.vector.tensor_add(sps[:, :, lo:lo + 128],
                                                 sps[:, :, lo:lo + 128], mka)
                        pt = ph2.tile([128, 2, 512], F16, tag="pt")
                        nc.scalar.activation(pt[:, :, lo:512], sps[:, :, lo:512],
                                             mybir.ActivationFunctionType.Exp,
                                             scale=float(SCALE))
                        if prev is not None:
                            emit_av(*prev)
                        prev = (pt, kb, lo)
                    emit_av(*prev)
                    # drain PSUM, then normalize by the ones-row sums.
                    ctxu = ph2c.tile([65, 2, 512], F32, tag="ctxu")
                    nc.vector.tensor_copy(ctxu[:, 0, :], cps[0][:])
                    nc.vector.tensor_copy(ctxu[:, 1, :], cps[1][:])
                    rs = ph2c.tile([1, 2, 512], F32, tag="rs")
                    nc.vector.tensor_copy(rs[:], ctxu[64:65, :, :])
                    rr = ph2c.tile([1, 2, 512], F32, tag="rr")
                    nc.vector.reciprocal_approx_fast(rr[:], rs[:])
                    rd = dram.tile([1, 2, 512], F32, tag="rd")
                    nc.gpsimd.dma_start(rd[:], rr[:])
                    bct = ph2c.tile([64, 2, 512], F32, tag="bct")
                    nc.gpsimd.dma_start(bct[:], rd[0:1, :, :].to_broadcast((64, 2, 512)))
                    for i in range(2):
                        nc.vector.tensor_mul(CN[hp][s][i * 64:i * 64 + 64, :],
                                             ctxu[0:64, i, :], bct[:, i, :])
                    # feed TensorE between hp groups (proj pieces of span s-1)
                    for _ in range(2):
                        if fillers:
                            fillers.pop(0)()
                for f in fillers:
                    f()

            def proj_pieces(s):
                def piece(qq, eo):
                    def run():
                        q0 = s * 512 + qq * 128
                        ops = psQ.tile([128, 512], F32, tag="pps")
                        for hp in range(4):
                            nc.tensor.matmul(ops[:],
                                             CN[hp][s][:, qq * 128:qq * 128 + 128],
                                             wpt[:, hp, eo * 512:(eo + 1) * 512],
                                             start=(hp == 0), stop=(hp == 3))
                        # bias is added host-side during the pair reduce.
                        ot = ph3.tile([128, 512], F16, tag="ot")
                        nc.vector.tensor_copy(ot[:], ops[:])
                        nc.sync.dma_start(out[q0:q0 + 128,
                                              eo * 512:(eo + 1) * 512], ot[:])
                    return run
                return [piece(qq, eo) for qq in range(4) for eo in range(2)]

            emit_qkv(0)
            emit_wp()
            emit_span(0)
            emit_qkv(1)
            emit_span(1, proj_pieces(0))
            emit_qkv(2)
            emit_span(2, proj_pieces(1))
            emit_qkv(3)
            emit_span(3, proj_pieces(2))
            for f in proj_pieces(3):
                f()
    nc.compile()
    return nc


_NC_CACHE = None


def _host_masks():
    """Static additive causal triangle for any diagonal 128x128 sub-block."""
    r = np.arange(128)[:, None]
    j = np.arange(128)[None, :]
    return np.where(j >= r, 0.0, -240.0).astype(np.float32)


def kernel(x, Wq, Wk, Wv, Wp, bp):
    global _NC_CACHE
    x = np.asarray(x, np.float32)
    Wq = np.asarray(Wq, np.float32)
    Wk = np.asarray(Wk, np.float32)
    Wv = np.asarray(Wv, np.float32)
    Wp = np.asarray(Wp, np.float32)
    bp = np.asarray(bp, np.float32)

    if _NC_CACHE is None:
        _NC_CACHE = _build_nc()
    nc = _NC_CACHE

    wqt = Wq.transpose(1, 0, 2).reshape(E, E).astype(np.float16)
    wkt = Wk.transpose(1, 0, 2).reshape(E, E).astype(np.float16)
    wvt = Wv.transpose(1, 0, 2).reshape(E, E).astype(np.float16)
    wp16 = Wp.astype(np.float16)
    mask = _host_masks()

    in_maps = []
    for c in range(8):
        b, h = c // 2, c % 2
        cols = slice(h * EH, (h + 1) * EH)
        in_maps.append({
            "xt": np.ascontiguousarray(x[b].T.astype(np.float16)),
            "wqt": np.ascontiguousarray(wqt[:, cols]),
            "wkt": np.ascontiguousarray(wkt[:, cols]),
            "wvt": np.ascontiguousarray(wvt[:, cols]),
            "wp": np.ascontiguousarray(wp16[cols, :]),
            "masks": mask,
        })

    trace = bool(os.environ.get("BASS_ATTN_TRACE"))
    res = run_bass_kernel_spmd(nc, in_maps, core_ids=list(range(8)), trace=trace)
    if trace and res.exec_time_ns is not None:
        print(f"HW exec time: {res.exec_time_ns} ns")
        for scope, cores in sorted((res.per_core_scope_times or {}).items()):
            print("scope", scope, cores)
        if res.instructions_and_trace:
            print("trace path:", res.instructions_and_trace[1])

    out = np.empty((B, S, E), np.float32)
    for b in range(B):
        out[b] = res.results[2 * b]["out"].astype(np.float32)
        out[b] += res.results[2 * b + 1]["out"].astype(np.float32)
        out[b] += bp
    return out
